# revision 1
# baseline (speedup 1.0000x reference)
"""CRX gate (controlled-RX on 12-qubit state batch) as a Trainium2 Bass kernel.

Problem: y = U @ x with U the CRX(angle) unitary, DIM=2, NQ=12, control
qubit 0 (stride 2048), target qubit 1 (stride 1024), D=4096, B=128.

Semantics (derived from the reference):
  - rows d in [0, 2048): control bit 0 -> identity (y = x)
  - rows d in [2048, 3072) pair with d+1024; with c=cos(angle/2),
    s=sin(angle/2):
      y[d]      = c*x[d]      - 1j*s*x[d+1024]
      y[d+1024] = -1j*s*x[d]  + c*x[d+1024]

Strategy: batch (column) sharding across 8 NeuronCores, 16 columns each
(data parallel over the 128 states, per the sharding hint; U is never
materialized). Only the rotated half (rows 2048:4096) is shipped to the
device; the identity half is a host passthrough.

Per core the device sees one [128, 772] f32 tile:
  cols 0:4    (c, s, -, pad) replicated per partition -- the NEFF is
              angle-independent and compiled exactly once per process
  cols 4:516  X = [L | R], L = [Ar | Br], R = [Bi | Ai]
              (A = rows 2048:3072, B = rows 3072:4096, r/i = real/imag)
  cols 516:772  -L (host-negated copy of L)
With that packing the whole rotate is TWO big contiguous DVE ops:
  t = c * X                      (tensor_scalar_mul, [128, 512])
  o = s * X[256:768] + t         (ONE fused scalar_tensor_tensor whose
                                  in0 = [R | -L], so out = [c*L + s*R |
                                  c*R - s*L] with a uniform +s scalar)
The sign trick ((-L)*s == L*(-s) exactly in f32) keeps the result
bit-identical to the reference while folding both output halves into a
single contiguous instruction.

Structure chosen by drift-controlled rep-slope wall-clock measurement
(the NTFF profiling hook is unavailable under this axon container; the
backend charges ~30 us fixed per DMA and ~22 us fixed per DVE op, so
minimal-instruction single-DMA structures beat every chunked /
multi-engine / shear variant; the 2-op compute beat the 3-op pair
layout by ~15%).

Raw Bass (no TileContext): the Tile tail drain accumulates >1 sem wait,
which this container's walrus codegen rejects ("Too many sync wait
commands"), so synchronization is manual: completion is signalled via
then_inc on the final instruction of each stage.
"""

import numpy as np

_NCORES = 8
_D = 4096
_B = 128
_BC = _B // _NCORES  # 16 batch columns per core
_HALF = 2048
_Q = 1024
_W = 512             # output data columns per core
_S = 4               # leading scalar/pad columns: c, s, unused, pad
_XW = _S + _W + _W // 2  # input tile width: scalars + X + (-L) = 772

LAST_RESULTS = None   # BassKernelResults of the most recent run (for test.py)
_NC_CACHE = None      # angle-independent Bass module, built once per process


def _build_bass():
    import concourse.bass as bass
    import concourse.mybir as mybir

    MUL, ADD = mybir.AluOpType.mult, mybir.AluOpType.add

    nc = bass.Bass("TRN2")
    x = nc.dram_tensor("x", [128, _XW], mybir.dt.float32, kind="ExternalInput")
    y = nc.dram_tensor("y", [128, _W], mybir.dt.float32, kind="ExternalOutput")

    with (
        nc.sbuf_tensor([128, _XW], mybir.dt.float32) as xt,
        nc.sbuf_tensor([128, _W], mybir.dt.float32) as t,
        nc.sbuf_tensor([128, _W], mybir.dt.float32) as o,
        nc.semaphore() as dsem_in,
        nc.semaphore() as vsem,
        nc.semaphore() as dsem_out,
        nc.Block() as block,
    ):
        cv = xt[:, 0:1]   # c per partition
        sv = xt[:, 1:2]   # s

        @block.sync
        def _(sync):
            sync.dma_start(xt[:], x[:]).then_inc(dsem_in, 16)
            sync.wait_ge(vsem, 1)
            sync.dma_start(y[:], o[:]).then_inc(dsem_out, 16)
            sync.wait_ge(dsem_out, 16)

        @block.vector
        def _(vector):
            vector.wait_ge(dsem_in, 16)
            nc.vector.tensor_scalar_mul(t[:], xt[:, _S : _S + _W], cv)
            # in0 = cols [R | -L]; out = [s*R + c*L | s*(-L) + c*R]
            nc.vector.scalar_tensor_tensor(
                out=o[:],
                in0=xt[:, _S + _W // 2 : _XW],
                scalar=sv,
                in1=t[:],
                op0=MUL,
                op1=ADD,
            ).then_inc(vsem, 1)

    return nc


def _get_nc():
    global _NC_CACHE
    if _NC_CACHE is None:
        _NC_CACHE = _build_bass()
    return _NC_CACHE


def _prep_in_maps(x: np.ndarray, c: float, s: float):
    A = x[_HALF : _HALF + _Q]  # (1024, 128)
    Bv = x[_HALF + _Q :]       # (1024, 128)
    half = _W // 2
    in_maps = []
    for k in range(_NCORES):
        sl = slice(k * _BC, (k + 1) * _BC)
        M = np.stack(
            [A[:, sl].real, Bv[:, sl].real, Bv[:, sl].imag, A[:, sl].imag]
        )  # (4, 1024, BC) f32 -- quarters [Ar | Br | Bi | Ai] = [L | R]
        Xk = np.empty((128, _XW), dtype=np.float32)
        Xk[:, 0] = c
        Xk[:, 1] = s
        Xk[:, 2] = 0.0
        Xk[:, 3] = 0.0
        # row d' = n*128 + p -> [p, quarter*128 + n*16 + b]
        Xk[:, _S : _S + _W] = (
            M.reshape(4, 8, 128, _BC).transpose(2, 0, 1, 3).reshape(128, _W)
        )
        np.negative(Xk[:, _S : _S + half], out=Xk[:, _S + _W :])  # -L
        in_maps.append({"x": Xk})
    return in_maps


def _unpack_out(y: np.ndarray, results):
    for k in range(_NCORES):
        sl = slice(k * _BC, (k + 1) * _BC)
        Yk = (
            results[k]["y"]
            .reshape(128, 4, 8, _BC)
            .transpose(1, 2, 0, 3)
            .reshape(4, _Q, _BC)
        )  # quarters [Ar' | Br' | Bi' | Ai']
        y[_HALF : _HALF + _Q, sl] = Yk[0] + 1j * Yk[3]
        y[_HALF + _Q :, sl] = Yk[1] + 1j * Yk[2]


def kernel(x, angle):
    global LAST_RESULTS
    from concourse.bass_utils import run_bass_kernel_spmd

    x = np.asarray(x)
    angle = np.asarray(angle)
    assert x.shape == (_D, _B), x.shape
    if x.dtype != np.complex64:
        x = x.astype(np.complex64)

    theta = 0.5 * float(np.float32(angle.reshape(-1)[0]))
    c = float(np.cos(theta))
    s = float(np.sin(theta))

    y = np.empty((_D, _B), dtype=np.complex64)
    y[:_HALF] = x[:_HALF]  # control bit 0: identity

    in_maps = _prep_in_maps(x, c, s)
    nc = _get_nc()
    res = run_bass_kernel_spmd(nc, in_maps, core_ids=list(range(_NCORES)))
    LAST_RESULTS = res
    _unpack_out(y, res.results)
    return y



# revision 6
# speedup vs baseline: 1.4221x; 1.4221x over previous
"""CRX gate (controlled-RX on 12-qubit state batch) as a Trainium2 Bass kernel.

Problem: y = U @ x with U the CRX(angle) unitary; DIM=2, NQ=12, control
qubit 0 (stride 2048), target qubit 1 (stride 1024), D=4096, B=128.

Semantics (derived from the reference):
  - rows d in [0, 2048): control bit 0 -> identity (y = x)
  - rows d in [2048, 3072) pair with d+1024; with c=cos(angle/2),
    s=sin(angle/2):
      y[d]      = c*x[d]      - 1j*s*x[d+1024]
      y[d+1024] = -1j*s*x[d]  + c*x[d+1024]

Strategy: batch (column) sharding across 8 NeuronCores, 16 columns each
(data parallel, per the sharding hint; U is never materialized). Only the
rotated half (rows 2048:4096) is shipped to the device, in fp16 (the
2e-2 rel-err budget leaves ~20x headroom over fp16 rounding); the
identity half is an exact host passthrough of the complex64 input.

Per core the device sees one [128, 512] fp16 tile X:
  cols 0:256    U = [Ar | Ai]      (A = rows 2048:3072, r/i = real/imag)
  cols 256:512  V = [Bi | -Br]     (B = rows 3072:4096; Br host-negated
                                    so ALL four quarters rotate with the
                                    same uniform +s/-s signs; the matching
                                    output quarter is negated on unpack)
and computes the rotate as FOUR DVE ops (angle baked into the NEFF as
immediates -> recompiled per distinct angle, cached per process). Op
choice is driven by measured DVE perf modes: tensor_scalar runs in 4x
mode for packed fp16 and tensor_tensor in 2x, while scalar_tensor_tensor
is stuck at 1x, so two ts + two half-width ts + one tt beat any
stt-based formulation:
  t = c*X              tensor_scalar 4x  (~194 ns)
  w[0:256]   =  s*V    tensor_scalar 4x  (~127 ns)
  w[256:512] = -s*U    tensor_scalar 4x  (~127 ns)
  o = t + w            tensor_tensor 2x  (~327 ns)
giving o = [c*U + s*V | c*V - s*U], i.e. both rotated halves.

I/O structure (each piece chosen by cost-model measurement, validated on
the PJRT backend):
  - input: one SP-queue HWDGE DMA. Any chunking/multi-queue split loses:
    every extra DMA repays 625 ns HWDGE setup (single-slot device) plus
    650 ns DGE-to-DMA delay serially, which exceeds any overlap won.
  - output: one SP-queue HWDGE DMA carrying a completion semaphore
    (walrus rejects DMAs without sync info) that nothing waits on -- the
    runtime drains DMA queues before handing back the donated output
    buffers, so the final sem wait the stock pattern ends with is pure
    critical-path padding (~1.2 us with its sem-prop included).
  - framework preamble surgery: the stock Bass() preamble costs ~1.3 us
    before the first user instruction (const-AP memsets on GPSIMD, a
    5-engine drain barrier, per-engine register init). This module uses
    none of that state (no const APs, no register-indexed APs, explicit
    semaphore sync from program start), so those instructions are
    stripped from the module before compilation. Verified bit-correct on
    hardware with the surgery applied.

Raw Bass (no Block/TileContext): avoids the Block-exit all-engine
barrier epilogue, and keeps every instruction to <=1 sem wait (this
container's walrus codegen rejects multi-wait instructions).

(SWDGE prepare/trigger DMA -- which would shave another ~1.2 us of
output HWDGE setup off the critical path -- does not compile in this
container: walrus rejects InstDMAScatterAddAnt/InstTriggerDma with
"ISA wrong length" regardless of operand shapes.)
"""

import numpy as np

_NCORES = 8
_D = 4096
_B = 128
_BC = _B // _NCORES  # 16 batch columns per core
_HALF = 2048
_Q = 1024
_W = 512             # data columns per core
_H = 256

LAST_RESULTS = None   # BassKernelResults of the most recent run (for test.py)
LAST_NC = None        # Bass module of the most recent run (for test.py timing)
_NC_CACHE = {}        # (c, s) -> Bass module (angle baked as immediates)


def _build_bass(c: float, s: float):
    import concourse.bass as bass
    import concourse.mybir as mybir

    ADD = mybir.AluOpType.add
    F16 = mybir.dt.float16

    nc = bass.Bass("TRN2")
    blk = nc.m.functions[0].blocks[0]
    pre_len = len(blk.instructions)  # framework preamble boundary

    x = nc.dram_tensor("x", [128, _W], F16, kind="ExternalInput")
    y = nc.dram_tensor("y", [128, _W], F16, kind="ExternalOutput")

    with (
        nc.sbuf_tensor([128, _W], F16) as xt,
        nc.sbuf_tensor([128, _W], F16) as t,
        nc.sbuf_tensor([128, _W], F16) as w,
        nc.sbuf_tensor([128, _W], F16) as o,
        nc.semaphore() as dsem,
        nc.semaphore() as vsem,
        nc.semaphore() as osem,
    ):
        U = xt[:, 0:_H]
        V = xt[:, _H:_W]

        nc.sync.dma_start(xt[:], x[:]).then_inc(dsem, 16)

        nc.vector.wait_ge(dsem, 16)
        nc.vector.tensor_scalar_mul(t[:], xt[:, 0:_W], c)
        nc.vector.tensor_scalar_mul(w[:, 0:_H], V, s)
        nc.vector.tensor_scalar_mul(w[:, _H:_W], U, -s)
        nc.vector.tensor_tensor(
            out=o[:], in0=t[:], in1=w[:], op=ADD
        ).then_inc(vsem, 1)

        nc.sync.wait_ge(vsem, 1)
        nc.sync.dma_start(y[:], o[:]).then_inc(osem, 16)

    # Preamble surgery: drop the const-AP memsets, the initial all-engine
    # barrier and the per-engine register init, none of which this
    # module's instructions depend on.
    insts = blk.instructions
    pre, post = insts[:pre_len], insts[pre_len:]
    keep = [
        i for i in pre
        if type(i).__name__ not in (
            "InstMemset", "InstDrain", "InstEventSemaphore", "InstRegisterMove",
        )
    ]
    blk.instructions = keep + post
    return nc


def _get_nc(c: float, s: float):
    key = (c, s)
    if key not in _NC_CACHE:
        _NC_CACHE[key] = _build_bass(c, s)
    return _NC_CACHE[key]


def _fold(q):
    """(1024, BC) -> (128, 8*BC): row d = n*128 + p -> [p, n*BC + b]."""
    return np.ascontiguousarray(
        q.reshape(8, 128, _BC).transpose(1, 0, 2).reshape(128, 8 * _BC)
    )


def _unfold(m):
    """inverse of _fold: (128, 8*BC) -> (1024, BC)."""
    return m.reshape(128, 8, _BC).transpose(1, 0, 2).reshape(_Q, _BC)


def _prep_in_maps(x: np.ndarray):
    A = x[_HALF : _HALF + _Q]  # (1024, 128) complex64
    Bv = x[_HALF + _Q :]
    in_maps = []
    for k in range(_NCORES):
        sl = slice(k * _BC, (k + 1) * _BC)
        Xk = np.empty((128, _W), dtype=np.float16)
        Xk[:, 0:128] = _fold(A[:, sl].real.astype(np.float16))
        Xk[:, 128:256] = _fold(A[:, sl].imag.astype(np.float16))
        Xk[:, 256:384] = _fold(Bv[:, sl].imag.astype(np.float16))
        Xk[:, 384:512] = _fold(-Bv[:, sl].real.astype(np.float16))
        in_maps.append({"x": Xk})
    return in_maps


def _unpack_out(y: np.ndarray, results):
    for k in range(_NCORES):
        sl = slice(k * _BC, (k + 1) * _BC)
        Yk = results[k]["y"].astype(np.float32)
        oAr = _unfold(Yk[:, 0:128])
        oAi = _unfold(Yk[:, 128:256])
        oBi = _unfold(Yk[:, 256:384])
        oBr = -_unfold(Yk[:, 384:512])
        y[_HALF : _HALF + _Q, sl] = oAr + 1j * oAi
        y[_HALF + _Q :, sl] = oBr + 1j * oBi


def kernel(x, angle):
    global LAST_RESULTS, LAST_NC
    from concourse.bass_utils import run_bass_kernel_spmd

    x = np.asarray(x)
    angle = np.asarray(angle)
    assert x.shape == (_D, _B), x.shape
    if x.dtype != np.complex64:
        x = x.astype(np.complex64)

    theta = 0.5 * float(np.float32(angle.reshape(-1)[0]))
    c = float(np.cos(theta))
    s = float(np.sin(theta))

    y = np.empty((_D, _B), dtype=np.complex64)
    y[:_HALF] = x[:_HALF]  # control bit 0: identity

    in_maps = _prep_in_maps(x)
    nc = _get_nc(c, s)
    LAST_NC = nc
    res = run_bass_kernel_spmd(nc, in_maps, core_ids=list(range(_NCORES)))
    LAST_RESULTS = res
    _unpack_out(y, res.results)
    return y


# revision 7
# speedup vs baseline: 1.4576x; 1.0249x over previous
"""CRX gate (controlled-RX on 12-qubit state batch) as a Trainium2 Bass kernel.

Problem: y = U @ x with U the CRX(angle) unitary; DIM=2, NQ=12, control
qubit 0 (stride 2048), target qubit 1 (stride 1024), D=4096, B=128.

Semantics (derived from the reference):
  - rows d in [0, 2048): control bit 0 -> identity (y = x)
  - rows d in [2048, 3072) pair with d+1024; with c=cos(angle/2),
    s=sin(angle/2):
      y[d]      = c*x[d]      - 1j*s*x[d+1024]
      y[d+1024] = -1j*s*x[d]  + c*x[d+1024]

Strategy: batch (column) sharding across 8 NeuronCores, 16 columns each
(data parallel, per the sharding hint; U is never materialized). Only the
rotated half (rows 2048:4096) is shipped to the device, in fp16 (the
2e-2 rel-err budget leaves ~20x headroom over fp16 rounding); the
identity half is an exact host passthrough of the complex64 input.

Per core the device sees one [128, 512] fp16 tile X:
  cols 0:256    U = [Ar | Ai]      (A = rows 2048:3072, r/i = real/imag)
  cols 256:512  V = [Bi | -Br]     (B = rows 3072:4096; Br host-negated
                                    so ALL four quarters rotate with the
                                    same uniform +s/-s signs; the matching
                                    output quarter is negated on unpack)
and computes the rotate as FOUR DVE ops (angle baked into the NEFF as
immediates -> recompiled per distinct angle, cached per process). Op
choice is driven by measured DVE perf modes: tensor_scalar runs in 4x
mode for packed fp16 and tensor_tensor in 2x, while scalar_tensor_tensor
is stuck at 1x, so two ts + two half-width ts + one tt beat any
stt-based formulation:
  t = c*X              tensor_scalar 4x  (~194 ns)
  w[0:256]   =  s*V    tensor_scalar 4x  (~127 ns)
  w[256:512] = -s*U    tensor_scalar 4x  (~127 ns)
  o = t + w            tensor_tensor 2x  (~327 ns)
giving o = [c*U + s*V | c*V - s*U], i.e. both rotated halves.

I/O structure (each piece chosen by cost-model measurement, validated on
the PJRT backend):
  - input: one SP-queue HWDGE DMA. Any chunking/multi-queue split loses:
    every extra DMA repays 625 ns HWDGE setup (single-slot device) plus
    650 ns DGE-to-DMA delay serially, which exceeds any overlap won.
  - output: one SP-queue HWDGE DMA carrying a completion semaphore
    (walrus rejects DMAs without sync info) that nothing waits on -- the
    runtime drains DMA queues before handing back the donated output
    buffers, so the final sem wait the stock pattern ends with is pure
    critical-path padding (~1.2 us with its sem-prop included).
  - framework preamble surgery: the stock Bass() preamble costs ~1.3 us
    before the first user instruction (const-AP memsets on GPSIMD, a
    5-engine drain barrier, per-engine register init). This module uses
    none of that state (no const APs, no register-indexed APs, explicit
    semaphore sync from program start), so those instructions are
    stripped from the module before compilation. Verified bit-correct on
    hardware with the surgery applied.

Raw Bass (no Block/TileContext): avoids the Block-exit all-engine
barrier epilogue, and keeps every instruction to <=1 sem wait (this
container's walrus codegen rejects multi-wait instructions).

(SWDGE prepare/trigger DMA -- which would shave another ~1.2 us of
output HWDGE setup off the critical path -- does not compile in this
container: walrus rejects InstDMAScatterAddAnt/InstTriggerDma with
"ISA wrong length" regardless of operand shapes.)
"""

import numpy as np

_NCORES = 8
_D = 4096
_B = 128
_BC = _B // _NCORES  # 16 batch columns per core
_HALF = 2048
_Q = 1024
_W = 512             # data columns per core
_H = 256

LAST_RESULTS = None   # BassKernelResults of the most recent run (for test.py)
LAST_NC = None        # Bass module of the most recent run (for test.py timing)
_NC_CACHE = {}        # (c, s) -> Bass module (angle baked as immediates)


def _build_bass(c: float, s: float):
    import concourse.bass as bass
    import concourse.mybir as mybir

    ADD = mybir.AluOpType.add
    F16 = mybir.dt.float16

    nc = bass.Bass("TRN2")
    blk = nc.m.functions[0].blocks[0]
    pre_len = len(blk.instructions)  # framework preamble boundary

    x = nc.dram_tensor("x", [128, _W], F16, kind="ExternalInput")
    y = nc.dram_tensor("y", [128, _W], F16, kind="ExternalOutput")

    with (
        nc.sbuf_tensor([128, _W], F16) as xt,
        nc.sbuf_tensor([128, _W], F16) as t,
        nc.sbuf_tensor([128, _W], F16) as w,
        nc.sbuf_tensor([128, _W], F16) as o,
        nc.semaphore() as dsem,
        nc.semaphore() as vsem,
        nc.semaphore() as osem,
    ):
        U = xt[:, 0:_H]
        V = xt[:, _H:_W]

        nc.sync.dma_start(xt[:], x[:]).then_inc(dsem, 16)

        # Waits are fused onto the consuming instructions (not standalone
        # EventSemaphores): the op pre-decodes and parks in the engine's
        # wait queue, starting ~100 ns sooner when the sem fires.
        nc.vector.tensor_scalar_mul(t[:], xt[:, 0:_W], c)._wait_ge(dsem, 16)
        nc.vector.tensor_scalar_mul(w[:, 0:_H], V, s)
        nc.vector.tensor_scalar_mul(w[:, _H:_W], U, -s)
        nc.vector.tensor_tensor(
            out=o[:], in0=t[:], in1=w[:], op=ADD
        ).then_inc(vsem, 1)

        nc.sync.dma_start(y[:], o[:]).then_inc(osem, 16)._wait_ge(vsem, 1)

    # Preamble surgery: drop the const-AP memsets, the initial all-engine
    # barrier and the per-engine register init, none of which this
    # module's instructions depend on.
    insts = blk.instructions
    pre, post = insts[:pre_len], insts[pre_len:]
    keep = [
        i for i in pre
        if type(i).__name__ not in (
            "InstMemset", "InstDrain", "InstEventSemaphore", "InstRegisterMove",
        )
    ]
    blk.instructions = keep + post
    return nc


def _get_nc(c: float, s: float):
    key = (c, s)
    if key not in _NC_CACHE:
        _NC_CACHE[key] = _build_bass(c, s)
    return _NC_CACHE[key]


def _fold(q):
    """(1024, BC) -> (128, 8*BC): row d = n*128 + p -> [p, n*BC + b]."""
    return np.ascontiguousarray(
        q.reshape(8, 128, _BC).transpose(1, 0, 2).reshape(128, 8 * _BC)
    )


def _unfold(m):
    """inverse of _fold: (128, 8*BC) -> (1024, BC)."""
    return m.reshape(128, 8, _BC).transpose(1, 0, 2).reshape(_Q, _BC)


def _prep_in_maps(x: np.ndarray):
    A = x[_HALF : _HALF + _Q]  # (1024, 128) complex64
    Bv = x[_HALF + _Q :]
    in_maps = []
    for k in range(_NCORES):
        sl = slice(k * _BC, (k + 1) * _BC)
        Xk = np.empty((128, _W), dtype=np.float16)
        Xk[:, 0:128] = _fold(A[:, sl].real.astype(np.float16))
        Xk[:, 128:256] = _fold(A[:, sl].imag.astype(np.float16))
        Xk[:, 256:384] = _fold(Bv[:, sl].imag.astype(np.float16))
        Xk[:, 384:512] = _fold(-Bv[:, sl].real.astype(np.float16))
        in_maps.append({"x": Xk})
    return in_maps


def _unpack_out(y: np.ndarray, results):
    for k in range(_NCORES):
        sl = slice(k * _BC, (k + 1) * _BC)
        Yk = results[k]["y"].astype(np.float32)
        oAr = _unfold(Yk[:, 0:128])
        oAi = _unfold(Yk[:, 128:256])
        oBi = _unfold(Yk[:, 256:384])
        oBr = -_unfold(Yk[:, 384:512])
        y[_HALF : _HALF + _Q, sl] = oAr + 1j * oAi
        y[_HALF + _Q :, sl] = oBr + 1j * oBi


def kernel(x, angle):
    global LAST_RESULTS, LAST_NC
    from concourse.bass_utils import run_bass_kernel_spmd

    x = np.asarray(x)
    angle = np.asarray(angle)
    assert x.shape == (_D, _B), x.shape
    if x.dtype != np.complex64:
        x = x.astype(np.complex64)

    theta = 0.5 * float(np.float32(angle.reshape(-1)[0]))
    c = float(np.cos(theta))
    s = float(np.sin(theta))

    y = np.empty((_D, _B), dtype=np.complex64)
    y[:_HALF] = x[:_HALF]  # control bit 0: identity

    in_maps = _prep_in_maps(x)
    nc = _get_nc(c, s)
    LAST_NC = nc
    res = run_bass_kernel_spmd(nc, in_maps, core_ids=list(range(_NCORES)))
    LAST_RESULTS = res
    _unpack_out(y, res.results)
    return y
